# revision 31
# baseline (speedup 1.0000x reference)
"""Trainium2 Bass kernel for MoE (noisy top-k gating, eval path) over 8 NeuronCores.

Strategy: expert-parallel. Each core owns E/8 = 2 experts (weights sharded on host).
Every core receives the full x and a per-core column-permuted w_gate so that its own
experts sit in gate columns 0..1 (top-k is permutation invariant). On device:

  1. Transpose x -> xT [I, B] via PE (128x128 identity matmuls).
  2. Gating: logits token-major via matmul, top-4 of 16 via 4x (reduce_max,
     is_equal, mask-subtract), softmax over the 4 maxima, local gate columns.
  3. Per token-block (512) x per local expert: FC1 (relu, bias via ACT), FC2
     (tanh via ACT + exp(10*t) via ACT), gate broadcast via K=1 ones-matmul,
     multiply-accumulate partial^T [O, B] (DVE).
  4. ReduceScatter(add) partial^T over 8 cores -> [O/8, B] shard, Ln, output.

Host unshard: concat o-shards -> [O, B], transpose -> [B, O].
"""

import ml_dtypes
import numpy as np

import concourse.bass as bass
import concourse.mybir as mybir
import concourse.tile as tile
from concourse import bacc
from concourse.alu_op_type import AluOpType
from concourse.bass_utils import run_bass_kernel_spmd
from concourse.masks import make_identity

F32 = mybir.dt.float32
F32R = mybir.dt.float32r  # 1 cycle/row on PE for N>=256 (vs 4 for fp32)
BF16 = mybir.dt.bfloat16
AF = mybir.ActivationFunctionType

B, I, H, O, E = 4096, 512, 1024, 512, 16
NCORES = 8
EL = E // NCORES          # 2 local experts per core
TOK = 512                 # token block (fp32 moving-operand max)
NBLK = B // TOK           # 8
KI = I // 128             # 4
KH = H // 128             # 8
KO = O // 128             # 4
NTT = B // 128            # 32 token tiles
TPB = TOK // 128          # 4 token tiles per block
OSH = O // NCORES         # 64 output rows per core after ReduceScatter

_BIG = 1.0e30


def _build_program():
    nc = bacc.Bacc(trn_type="TRN2", num_devices=NCORES)

    x_d = nc.dram_tensor("x", [B, I], F32, kind="ExternalInput")
    wg_d = nc.dram_tensor("wg", [I, E], F32, kind="ExternalInput")
    w1_d = nc.dram_tensor("w1", [EL, I, H], F32, kind="ExternalInput")
    b1_d = nc.dram_tensor("b1", [EL, H], F32, kind="ExternalInput")
    w2_d = nc.dram_tensor("w2", [EL, H, O], F32, kind="ExternalInput")
    b2_d = nc.dram_tensor("b2", [EL, O], F32, kind="ExternalInput")
    out_d = nc.dram_tensor("out", [OSH, B], F32, kind="ExternalOutput")

    with tile.TileContext(nc) as tc:
        with (
            tc.tile_pool(name="const", bufs=1) as constp,
            tc.tile_pool(name="xtp", bufs=1) as xtp,
            tc.tile_pool(name="wp", bufs=1) as wp,
            tc.tile_pool(name="gatep", bufs=1) as gatep,
            tc.tile_pool(name="dram", bufs=1, space="DRAM") as dramp,
        ):
            ident = constp.tile([128, 128], F32)
            make_identity(nc, ident[:])
            ones1f = constp.tile([1, 128], F32)
            nc.vector.memset(ones1f[:], 1.0)
            ones1 = constp.tile([1, 128], F32R)
            nc.vector.tensor_copy(ones1[:], ones1f[:])

            # ---------- weights (resident, bf16 from host) ----------
            # scalar-engine DMA queue so x tiles (sync queue) aren't stuck
            # behind the big weight transfers; per-expert chunks so expert 0's
            # FC1 can start as soon as its slice lands
            w1s = wp.tile([128, EL, KI, H], F32R)  # w1s[p,e,ki,h] = W1[e, ki*128+p, h]
            w2s = wp.tile([128, EL, KH, O], F32R)  # w2s[p,e,kh,o] = W2[e, kh*128+p, o]
            for e in range(EL):
                nc.scalar.dma_start(
                    w1s[:, e], w1_d[e].rearrange("(ki p) h -> p ki h", p=128).bitcast(F32R)
                )
                nc.scalar.dma_start(
                    w2s[:, e], w2_d[e].rearrange("(kh p) o -> p kh o", p=128).bitcast(F32R)
                )
            b1T = wp.tile([128, EL, KH], F32)     # b1T[p,e,hi] = b1[e, hi*128+p]
            nc.scalar.dma_start(b1T[:], b1_d.rearrange("e (kh p) -> p e kh", p=128))
            b2T = wp.tile([128, EL, KO], F32)
            nc.scalar.dma_start(b2T[:], b2_d.rearrange("e (ko p) -> p e ko", p=128))

            # persistent xT and local gates
            xT = []
            for ki in range(KI):
                t_ = xtp.tile([128, B], F32R, name=f"xT{ki}")
                xT.append(t_)
            gloc = gatep.tile([128, NTT, EL], F32)

            # ---------- scoped: x transpose + gating ----------
            with (
                tc.tile_pool(name="scratch", bufs=1) as scr,
                tc.tile_pool(name="psum_s", bufs=2, space="PSUM") as psum_s,
            ):
                wgs = scr.tile([128, KI, E], F32)  # wgs[p,ki,e] = wg[ki*128+p, e]
                nc.sync.dma_start(wgs[:], wg_d.rearrange("(ki p) e -> p ki e", p=128))

                # logits, token-major packed [128, NTT, E].  The gating matmul
                # must be EXACT fp32 (top-k flips are catastrophic), so evac
                # each transposed block twice: fp32r into resident xT for the
                # expert FCs, fp32 into a transient block for the logits.
                Lg = scr.tile([128, NTT, E], F32)
                for t in range(NTT):
                    x_tile = scr.tile([128, I], F32, tag="x_in", bufs=3)
                    nc.sync.dma_start(x_tile[:], x_d[t * 128:(t + 1) * 128, :])
                    xtg = scr.tile([128, KI, 128], F32, tag="xtg", bufs=3)
                    for ki in range(KI):
                        pt = psum_s.tile([128, 128], F32, tag="ptr")
                        nc.tensor.transpose(
                            pt[:], x_tile[:, ki * 128:(ki + 1) * 128], ident[:]
                        )
                        if (t * KI + ki) % 2 == 0:
                            nc.scalar.activation(
                                xT[ki][:, t * 128:(t + 1) * 128], pt[:], AF.Copy
                            )
                            nc.vector.tensor_copy(xtg[:, ki, :], pt[:])
                        else:
                            nc.vector.tensor_copy(
                                xT[ki][:, t * 128:(t + 1) * 128], pt[:]
                            )
                            nc.scalar.activation(xtg[:, ki, :], pt[:], AF.Copy)
                    pg = psum_s.tile([128, E], F32, tag="pg")
                    for ki in range(KI):
                        nc.tensor.matmul(
                            pg[:],
                            xtg[:, ki, :],
                            wgs[:, ki, :],
                            start=(ki == 0),
                            stop=(ki == KI - 1),
                        )
                    nc.vector.tensor_copy(Lg[:, t, :], pg[:])

                # top-4 of 16 per token
                mx = [scr.tile([128, NTT, 1], F32, name=f"mx{j}") for j in range(4)]
                eq = [scr.tile([128, NTT, E], F32, name=f"eq{j}") for j in range(4)]
                for j in range(4):
                    nc.vector.tensor_reduce(
                        mx[j][:], Lg[:], mybir.AxisListType.X, AluOpType.max
                    )
                    nc.vector.tensor_tensor(
                        eq[j][:], Lg[:], mx[j].to_broadcast([128, NTT, E]),
                        AluOpType.is_equal,
                    )
                    if j < 3:
                        # Lg = (eq * -BIG) + Lg  -- knock out the found max
                        nc.vector.scalar_tensor_tensor(
                            Lg[:], eq[j][:], -_BIG, Lg[:],
                            AluOpType.mult, AluOpType.add,
                        )

                # softmax over the 4 maxima: g_j = exp(m_j - m_0) / sum
                ex = [scr.tile([128, NTT, 1], F32, name=f"ex{j}") for j in range(4)]
                for j in range(1, 4):
                    nc.vector.tensor_sub(ex[j][:], mx[j][:], mx[0][:])
                    nc.scalar.activation(ex[j][:], ex[j][:], AF.Exp)
                denom = scr.tile([128, NTT, 1], F32)
                nc.vector.tensor_add(denom[:], ex[1][:], ex[2][:])
                nc.vector.tensor_add(denom[:], denom[:], ex[3][:])
                nc.vector.tensor_scalar_add(denom[:], denom[:], 1.0)
                rec = scr.tile([128, NTT, 1], F32)
                nc.vector.reciprocal(rec[:], denom[:])
                gj = [scr.tile([128, NTT, 1], F32, name=f"gj{j}") for j in range(4)]
                nc.vector.tensor_copy(gj[0][:], rec[:])
                for j in range(1, 4):
                    nc.vector.tensor_mul(gj[j][:], ex[j][:], rec[:])

                # local dense gates (this core's experts are gate cols 0..EL-1)
                tmpg = scr.tile([128, NTT, EL], F32)
                nc.vector.tensor_tensor(
                    gloc[:], eq[0][:, :, :EL], gj[0].to_broadcast([128, NTT, EL]),
                    AluOpType.mult,
                )
                for j in range(1, 4):
                    nc.vector.tensor_tensor(
                        tmpg[:], eq[j][:, :, :EL], gj[j].to_broadcast([128, NTT, EL]),
                        AluOpType.mult,
                    )
                    nc.vector.tensor_add(gloc[:], gloc[:], tmpg[:])

            # ---------- main loop: MLP + combine ----------
            # asymmetric token split (6 blocks / 2 blocks): the big first
            # ReduceScatter overlaps the tail compute, the small second one
            # is the only serial tail
            BLKA = 6
            B_A = BLKA * TOK
            B_B = B - B_A
            partial_a = dramp.tile([O, B_A], F32)
            partial_b = dramp.tile([O, B_B], F32)
            pviews = [
                partial_a.rearrange("(oi p) b -> p oi b", p=128),
                partial_b.rearrange("(oi p) b -> p oi b", p=128),
            ]

            with (
                tc.tile_pool(name="work", bufs=2) as work,
                tc.tile_pool(name="psum_m", bufs=2, space="PSUM") as psum_m,
            ):
                for blk in range(NBLK):
                    t0 = blk * TOK
                    part_blk = work.tile([128, KO, TOK], F32, tag="part", bufs=2)
                    for e in range(EL):
                        # assemble gate row [1, TOK] via per-token-tile PE transpose
                        grow = work.tile([1, TOK], F32R, tag="grow", bufs=2)
                        for j in range(TPB):
                            ptg = psum_m.tile([1, 128], F32, tag="ptg")
                            nc.tensor.transpose(
                                ptg[:], gloc[:, blk * TPB + j, e:e + 1], ident[:]
                            )
                            nc.vector.tensor_copy(
                                grow[:, j * 128:(j + 1) * 128], ptg[:]
                            )
                        # broadcast gate row across 128 partitions (K=1 matmul)
                        pgb = psum_m.tile([128, TOK], F32, tag="pgb")
                        nc.tensor.matmul(
                            pgb[:], ones1[:], grow[:],
                            start=True, stop=True,
                        )

                        hT = work.tile([128, KH, TOK], F32R, tag="hT", bufs=2)
                        for hi in range(KH):
                            ph = psum_m.tile([128, TOK], F32, tag="ph")
                            for ki in range(KI):
                                nc.tensor.matmul(
                                    ph[:],
                                    w1s[:, e, ki, hi * 128:(hi + 1) * 128],
                                    xT[ki][:, t0:t0 + TOK],
                                    start=(ki == 0),
                                    stop=(ki == KI - 1),
                                )
                            nc.scalar.activation(
                                hT[:, hi, :], ph[:], AF.Relu,
                                bias=b1T[:, e, hi:hi + 1],
                            )
                        for oi in range(KO):
                            po = psum_m.tile([128, TOK], F32, tag="po")
                            for hi in range(KH):
                                nc.tensor.matmul(
                                    po[:],
                                    w2s[:, e, hi, oi * 128:(oi + 1) * 128],
                                    hT[:, hi, :],
                                    start=(hi == 0),
                                    stop=(hi == KH - 1),
                                )
                            tt = work.tile([128, TOK], F32, tag="tt", bufs=2)
                            nc.scalar.activation(
                                tt[:], po[:], AF.Tanh, bias=b2T[:, e, oi:oi + 1]
                            )
                            nc.scalar.activation(tt[:], tt[:], AF.Exp, scale=10.0)
                            # multiply by broadcast gate (read PSUM directly)
                            if e == 0:
                                nc.vector.tensor_tensor(
                                    part_blk[:, oi, :], tt[:], pgb[:],
                                    AluOpType.mult,
                                )
                            else:
                                nc.vector.tensor_tensor(
                                    tt[:], tt[:], pgb[:], AluOpType.mult
                                )
                                nc.vector.tensor_add(
                                    part_blk[:, oi, :], part_blk[:, oi, :], tt[:]
                                )
                    half = 0 if blk < BLKA else 1
                    th = t0 - half * B_A
                    nc.sync.dma_start(
                        pviews[half][:, :, th:th + TOK], part_blk[:]
                    )
                # Emit both collectives AFTER the block loop: the gpsimd queue
                # stalls at a collective until it completes, so any Tile
                # bookkeeping emitted later on gpsimd (which the PE stream can
                # depend on) must not sit behind it.  Deps still let RS-a
                # start as soon as the partial_a DMAs (block BLKA-1) land,
                # overlapping the remaining blocks' compute.
                rs_out_a = dramp.tile([OSH, B_A], F32)
                nc.gpsimd.collective_compute(
                    "ReduceScatter",
                    AluOpType.add,
                    replica_groups=[list(range(NCORES))],
                    ins=[partial_a[:]],
                    outs=[rs_out_a[:]],
                )

                rs_out_b = dramp.tile([OSH, B_B], F32)
                nc.gpsimd.collective_compute(
                    "ReduceScatter",
                    AluOpType.add,
                    replica_groups=[list(range(NCORES))],
                    ins=[partial_b[:]],
                    outs=[rs_out_b[:]],
                )

                # ---------- log + output ----------
                # scalar-queue DMA + private tag: fin waits on the collective,
                # so it must not sit in the sync queue (or share slots) ahead
                # of the tail blocks' partial DMAs
                for off, width, rs_o, nm in (
                    (0, B_A, rs_out_a, "fina"),
                    (B_A, B_B, rs_out_b, "finb"),
                ):
                    fin = work.tile([OSH, width], F32, tag="fin", bufs=1, name=nm)
                    nc.scalar.dma_start(fin[:], rs_o[:])
                    nc.scalar.activation(fin[:], fin[:], AF.Ln)
                    nc.scalar.dma_start(out_d[:, off:off + width], fin[:])

    nc.compile()
    return nc


_NC_CACHE = None
LAST_RESULT = None


def _get_nc():
    global _NC_CACHE
    if _NC_CACHE is None:
        _NC_CACHE = _build_program()
    return _NC_CACHE


def kernel(x, w_gate, W1, b1, W2, b2, k, trace=False):
    global LAST_RESULT
    assert int(k) == 4
    x = np.ascontiguousarray(np.asarray(x, dtype=np.float32))
    w_gate = np.asarray(w_gate, dtype=np.float32)
    W1 = np.asarray(W1, dtype=np.float32)
    b1 = np.asarray(b1, dtype=np.float32)
    W2 = np.asarray(W2, dtype=np.float32)
    b2 = np.asarray(b2, dtype=np.float32)

    nc = _get_nc()
    in_maps = []
    for c in range(NCORES):
        mine = [EL * c + j for j in range(EL)]
        rest = [e for e in range(E) if e not in mine]
        perm = mine + rest
        in_maps.append({
            "x": x,
            "wg": np.ascontiguousarray(w_gate[:, perm]),
            "w1": np.ascontiguousarray(W1[mine]),
            "b1": np.ascontiguousarray(b1[mine]),
            "w2": np.ascontiguousarray(W2[mine]),
            "b2": np.ascontiguousarray(b2[mine]),
        })

    res = run_bass_kernel_spmd(
        nc, in_maps, core_ids=list(range(NCORES)), trace=trace
    )
    LAST_RESULT = res
    outT = np.concatenate([res.results[c]["out"] for c in range(NCORES)], axis=0)
    return np.ascontiguousarray(outT.T)


# revision 32
# speedup vs baseline: 1.0397x; 1.0397x over previous
"""Trainium2 Bass kernel for MoE (noisy top-k gating, eval path) over 8 NeuronCores.

Strategy: expert-parallel. Each core owns E/8 = 2 experts (weights sharded on host).
Every core receives the full x and a per-core column-permuted w_gate so that its own
experts sit in gate columns 0..1 (top-k is permutation invariant). On device:

  1. Transpose x -> xT [I, B] via PE (128x128 identity matmuls).
  2. Gating: logits token-major via matmul, top-4 of 16 via 4x (reduce_max,
     is_equal, mask-subtract), softmax over the 4 maxima, local gate columns.
  3. Per token-block (512) x per local expert: FC1 (relu, bias via ACT), FC2
     (tanh via ACT + exp(10*t) via ACT), gate broadcast via K=1 ones-matmul,
     multiply-accumulate partial^T [O, B] (DVE).
  4. ReduceScatter(add) partial^T over 8 cores -> [O/8, B] shard, Ln, output.

Host unshard: concat o-shards -> [O, B], transpose -> [B, O].
"""

import ml_dtypes
import numpy as np

import concourse.bass as bass
import concourse.mybir as mybir
import concourse.tile as tile
from concourse import bacc
from concourse.alu_op_type import AluOpType
from concourse.bass_utils import run_bass_kernel_spmd
from concourse.masks import make_identity

F32 = mybir.dt.float32
F32R = mybir.dt.float32r  # 1 cycle/row on PE for N>=256 (vs 4 for fp32)
BF16 = mybir.dt.bfloat16
AF = mybir.ActivationFunctionType

B, I, H, O, E = 4096, 512, 1024, 512, 16
NCORES = 8
EL = E // NCORES          # 2 local experts per core
TOK = 512                 # token block (fp32 moving-operand max)
NBLK = B // TOK           # 8
KI = I // 128             # 4
KH = H // 128             # 8
KO = O // 128             # 4
NTT = B // 128            # 32 token tiles
TPB = TOK // 128          # 4 token tiles per block
OSH = O // NCORES         # 64 output rows per core after ReduceScatter

_BIG = 1.0e30


def _build_program():
    nc = bacc.Bacc(trn_type="TRN2", num_devices=NCORES)

    x_d = nc.dram_tensor("x", [B, I], F32, kind="ExternalInput")
    wg_d = nc.dram_tensor("wg", [I, E], F32, kind="ExternalInput")
    w1_d = nc.dram_tensor("w1", [EL, I, H], F32, kind="ExternalInput")
    b1_d = nc.dram_tensor("b1", [EL, H], F32, kind="ExternalInput")
    w2_d = nc.dram_tensor("w2", [EL, H, O], F32, kind="ExternalInput")
    b2_d = nc.dram_tensor("b2", [EL, O], F32, kind="ExternalInput")
    out_d = nc.dram_tensor("out", [OSH, B], F32, kind="ExternalOutput")

    with tile.TileContext(nc) as tc:
        with (
            tc.tile_pool(name="const", bufs=1) as constp,
            tc.tile_pool(name="xtp", bufs=1) as xtp,
            tc.tile_pool(name="wp", bufs=1) as wp,
            tc.tile_pool(name="gatep", bufs=1) as gatep,
            tc.tile_pool(name="dram", bufs=1, space="DRAM") as dramp,
        ):
            ident = constp.tile([128, 128], F32)
            make_identity(nc, ident[:])
            ones1f = constp.tile([1, 128], F32)
            nc.vector.memset(ones1f[:], 1.0)
            ones1 = constp.tile([1, 128], F32R)
            nc.vector.tensor_copy(ones1[:], ones1f[:])

            # ---------- weights (resident, bf16 from host) ----------
            # scalar-engine DMA queue so x tiles (sync queue) aren't stuck
            # behind the big weight transfers; per-expert chunks so expert 0's
            # FC1 can start as soon as its slice lands
            w1s = wp.tile([128, EL, KI, H], F32R)  # w1s[p,e,ki,h] = W1[e, ki*128+p, h]
            w2s = wp.tile([128, EL, KH, O], F32R)  # w2s[p,e,kh,o] = W2[e, kh*128+p, o]
            for e in range(EL):
                nc.scalar.dma_start(
                    w1s[:, e], w1_d[e].rearrange("(ki p) h -> p ki h", p=128).bitcast(F32R)
                )
                nc.scalar.dma_start(
                    w2s[:, e], w2_d[e].rearrange("(kh p) o -> p kh o", p=128).bitcast(F32R)
                )
            b1T = wp.tile([128, EL, KH], F32)     # b1T[p,e,hi] = b1[e, hi*128+p]
            nc.scalar.dma_start(b1T[:], b1_d.rearrange("e (kh p) -> p e kh", p=128))
            b2T = wp.tile([128, EL, KO], F32)
            nc.scalar.dma_start(b2T[:], b2_d.rearrange("e (ko p) -> p e ko", p=128))

            # persistent xT and local gates
            xT = []
            for ki in range(KI):
                t_ = xtp.tile([128, B], F32R, name=f"xT{ki}")
                xT.append(t_)
            gloc = gatep.tile([128, NTT, EL], F32)

            # ---------- scoped: x transpose + gating ----------
            with (
                tc.tile_pool(name="scratch", bufs=1) as scr,
                tc.tile_pool(name="psum_s", bufs=2, space="PSUM") as psum_s,
            ):
                wgs = scr.tile([128, KI, E], F32)  # wgs[p,ki,e] = wg[ki*128+p, e]
                nc.sync.dma_start(wgs[:], wg_d.rearrange("(ki p) e -> p ki e", p=128))

                # logits, token-major packed [128, NTT, E].  The gating matmul
                # must be EXACT fp32 (top-k flips are catastrophic), so evac
                # each transposed block twice: fp32r into resident xT for the
                # expert FCs, fp32 into a transient block for the logits.
                Lg = scr.tile([128, NTT, E], F32)
                for t in range(NTT):
                    x_tile = scr.tile([128, I], F32, tag="x_in", bufs=3)
                    nc.sync.dma_start(x_tile[:], x_d[t * 128:(t + 1) * 128, :])
                    xtg = scr.tile([128, KI, 128], F32, tag="xtg", bufs=3)
                    for ki in range(KI):
                        pt = psum_s.tile([128, 128], F32, tag="ptr")
                        nc.tensor.transpose(
                            pt[:], x_tile[:, ki * 128:(ki + 1) * 128], ident[:]
                        )
                        if (t * KI + ki) % 2 == 0:
                            nc.scalar.activation(
                                xT[ki][:, t * 128:(t + 1) * 128], pt[:], AF.Copy
                            )
                            nc.vector.tensor_copy(xtg[:, ki, :], pt[:])
                        else:
                            nc.vector.tensor_copy(
                                xT[ki][:, t * 128:(t + 1) * 128], pt[:]
                            )
                            nc.scalar.activation(xtg[:, ki, :], pt[:], AF.Copy)
                    pg = psum_s.tile([128, E], F32, tag="pg")
                    for ki in range(KI):
                        nc.tensor.matmul(
                            pg[:],
                            xtg[:, ki, :],
                            wgs[:, ki, :],
                            start=(ki == 0),
                            stop=(ki == KI - 1),
                        )
                    nc.vector.tensor_copy(Lg[:, t, :], pg[:])

                # top-4 of 16 per token
                mx = [scr.tile([128, NTT, 1], F32, name=f"mx{j}") for j in range(4)]
                eq = [scr.tile([128, NTT, E], F32, name=f"eq{j}") for j in range(4)]
                for j in range(4):
                    nc.vector.tensor_reduce(
                        mx[j][:], Lg[:], mybir.AxisListType.X, AluOpType.max
                    )
                    nc.vector.tensor_tensor(
                        eq[j][:], Lg[:], mx[j].to_broadcast([128, NTT, E]),
                        AluOpType.is_equal,
                    )
                    if j < 3:
                        # Lg = (eq * -BIG) + Lg  -- knock out the found max
                        nc.vector.scalar_tensor_tensor(
                            Lg[:], eq[j][:], -_BIG, Lg[:],
                            AluOpType.mult, AluOpType.add,
                        )

                # softmax over the 4 maxima: g_j = exp(m_j - m_0) / sum
                ex = [scr.tile([128, NTT, 1], F32, name=f"ex{j}") for j in range(4)]
                for j in range(1, 4):
                    nc.vector.tensor_sub(ex[j][:], mx[j][:], mx[0][:])
                    nc.scalar.activation(ex[j][:], ex[j][:], AF.Exp)
                denom = scr.tile([128, NTT, 1], F32)
                nc.vector.tensor_add(denom[:], ex[1][:], ex[2][:])
                nc.vector.tensor_add(denom[:], denom[:], ex[3][:])
                nc.vector.tensor_scalar_add(denom[:], denom[:], 1.0)
                rec = scr.tile([128, NTT, 1], F32)
                nc.vector.reciprocal(rec[:], denom[:])
                gj = [scr.tile([128, NTT, 1], F32, name=f"gj{j}") for j in range(4)]
                nc.vector.tensor_copy(gj[0][:], rec[:])
                for j in range(1, 4):
                    nc.vector.tensor_mul(gj[j][:], ex[j][:], rec[:])

                # local dense gates (this core's experts are gate cols 0..EL-1)
                tmpg = scr.tile([128, NTT, EL], F32)
                nc.vector.tensor_tensor(
                    gloc[:], eq[0][:, :, :EL], gj[0].to_broadcast([128, NTT, EL]),
                    AluOpType.mult,
                )
                for j in range(1, 4):
                    nc.vector.tensor_tensor(
                        tmpg[:], eq[j][:, :, :EL], gj[j].to_broadcast([128, NTT, EL]),
                        AluOpType.mult,
                    )
                    nc.vector.tensor_add(gloc[:], gloc[:], tmpg[:])

            # ---------- main loop: MLP + combine ----------
            # asymmetric token split (6 blocks / 2 blocks): the big first
            # ReduceScatter overlaps the tail compute, the small second one
            # is the only serial tail
            BLKA = 6
            B_A = BLKA * TOK
            B_B = B - B_A
            partial_a = dramp.tile([O, B_A], F32)
            partial_b = dramp.tile([O, B_B], F32)
            pviews = [
                partial_a.rearrange("(oi p) b -> p oi b", p=128),
                partial_b.rearrange("(oi p) b -> p oi b", p=128),
            ]

            with (
                tc.tile_pool(name="work", bufs=2) as work,
                tc.tile_pool(name="psum_m", bufs=2, space="PSUM") as psum_m,
            ):
                for blk in range(NBLK):
                    t0 = blk * TOK
                    part_blk = work.tile([128, KO, TOK], F32, tag="part", bufs=2)
                    for e in range(EL):
                        # assemble gate row [1, TOK] via per-token-tile PE transpose
                        grow = work.tile([1, TOK], F32R, tag="grow", bufs=2)
                        for j in range(TPB):
                            ptg = psum_m.tile([1, 128], F32, tag="ptg")
                            nc.tensor.transpose(
                                ptg[:], gloc[:, blk * TPB + j, e:e + 1], ident[:]
                            )
                            nc.vector.tensor_copy(
                                grow[:, j * 128:(j + 1) * 128], ptg[:]
                            )
                        # broadcast gate row across 128 partitions (K=1 matmul)
                        pgb = psum_m.tile([128, TOK], F32, tag="pgb")
                        nc.tensor.matmul(
                            pgb[:], ones1[:], grow[:],
                            start=True, stop=True,
                        )

                        hT = work.tile([128, KH, TOK], F32R, tag="hT", bufs=2)
                        for hi in range(KH):
                            ph = psum_m.tile([128, TOK], F32, tag="ph")
                            for ki in range(KI):
                                nc.tensor.matmul(
                                    ph[:],
                                    w1s[:, e, ki, hi * 128:(hi + 1) * 128],
                                    xT[ki][:, t0:t0 + TOK],
                                    start=(ki == 0),
                                    stop=(ki == KI - 1),
                                )
                            nc.scalar.activation(
                                hT[:, hi, :], ph[:], AF.Relu,
                                bias=b1T[:, e, hi:hi + 1],
                            )
                        for oi in range(KO):
                            po = psum_m.tile([128, TOK], F32, tag="po")
                            for hi in range(KH):
                                nc.tensor.matmul(
                                    po[:],
                                    w2s[:, e, hi, oi * 128:(oi + 1) * 128],
                                    hT[:, hi, :],
                                    start=(hi == 0),
                                    stop=(hi == KH - 1),
                                )
                            tt = work.tile([128, TOK], F32, tag="tt", bufs=2)
                            nc.scalar.activation(
                                tt[:], po[:], AF.Tanh, bias=b2T[:, e, oi:oi + 1]
                            )
                            nc.scalar.activation(tt[:], tt[:], AF.Exp, scale=10.0)
                            # multiply by broadcast gate (read PSUM directly)
                            if e == 0:
                                nc.vector.tensor_tensor(
                                    part_blk[:, oi, :], tt[:], pgb[:],
                                    AluOpType.mult,
                                )
                            else:
                                nc.vector.tensor_tensor(
                                    tt[:], tt[:], pgb[:], AluOpType.mult
                                )
                                nc.vector.tensor_add(
                                    part_blk[:, oi, :], part_blk[:, oi, :], tt[:]
                                )
                    half = 0 if blk < BLKA else 1
                    th = t0 - half * B_A
                    nc.sync.dma_start(
                        pviews[half][:, :, th:th + TOK], part_blk[:]
                    )
                # Emit both collectives AFTER the block loop: the gpsimd queue
                # stalls at a collective until it completes, so any Tile
                # bookkeeping emitted later on gpsimd (which the PE stream can
                # depend on) must not sit behind it.  Deps still let RS-a
                # start as soon as the partial_a DMAs (block BLKA-1) land,
                # overlapping the remaining blocks' compute.
                rs_out_a = dramp.tile([OSH, B_A], F32)
                nc.gpsimd.collective_compute(
                    "ReduceScatter",
                    AluOpType.add,
                    replica_groups=[list(range(NCORES))],
                    ins=[partial_a[:]],
                    outs=[rs_out_a[:]],
                )

                rs_out_b = dramp.tile([OSH, B_B], F32)
                nc.gpsimd.collective_compute(
                    "ReduceScatter",
                    AluOpType.add,
                    replica_groups=[list(range(NCORES))],
                    ins=[partial_b[:]],
                    outs=[rs_out_b[:]],
                )

                # ---------- log + output ----------
                for off, width, rs_o, nm in (
                    (0, B_A, rs_out_a, "fina"),
                    (B_A, B_B, rs_out_b, "finb"),
                ):
                    fin = work.tile([OSH, width], F32, tag="part", name=nm)
                    nc.sync.dma_start(fin[:], rs_o[:])
                    nc.scalar.activation(fin[:], fin[:], AF.Ln)
                    nc.sync.dma_start(out_d[:, off:off + width], fin[:])

    nc.compile()
    return nc


_NC_CACHE = None
LAST_RESULT = None


def _get_nc():
    global _NC_CACHE
    if _NC_CACHE is None:
        _NC_CACHE = _build_program()
    return _NC_CACHE


def kernel(x, w_gate, W1, b1, W2, b2, k, trace=False):
    global LAST_RESULT
    assert int(k) == 4
    x = np.ascontiguousarray(np.asarray(x, dtype=np.float32))
    w_gate = np.asarray(w_gate, dtype=np.float32)
    W1 = np.asarray(W1, dtype=np.float32)
    b1 = np.asarray(b1, dtype=np.float32)
    W2 = np.asarray(W2, dtype=np.float32)
    b2 = np.asarray(b2, dtype=np.float32)

    nc = _get_nc()
    in_maps = []
    for c in range(NCORES):
        mine = [EL * c + j for j in range(EL)]
        rest = [e for e in range(E) if e not in mine]
        perm = mine + rest
        in_maps.append({
            "x": x,
            "wg": np.ascontiguousarray(w_gate[:, perm]),
            "w1": np.ascontiguousarray(W1[mine]),
            "b1": np.ascontiguousarray(b1[mine]),
            "w2": np.ascontiguousarray(W2[mine]),
            "b2": np.ascontiguousarray(b2[mine]),
        })

    res = run_bass_kernel_spmd(
        nc, in_maps, core_ids=list(range(NCORES)), trace=trace
    )
    LAST_RESULT = res
    outT = np.concatenate([res.results[c]["out"] for c in range(NCORES)], axis=0)
    return np.ascontiguousarray(outT.T)


# revision 33
# speedup vs baseline: 1.0804x; 1.0391x over previous
"""Trainium2 Bass kernel for MoE (noisy top-k gating, eval path) over 8 NeuronCores.

Strategy: expert-parallel. Each core owns E/8 = 2 experts (weights sharded on host).
Every core receives the full x and a per-core column-permuted w_gate so that its own
experts sit in gate columns 0..1 (top-k is permutation invariant). On device:

  1. Transpose x -> xT [I, B] via PE (128x128 identity matmuls).
  2. Gating: logits token-major via matmul, top-4 of 16 via 4x (reduce_max,
     is_equal, mask-subtract), softmax over the 4 maxima, local gate columns.
  3. Per token-block (512) x per local expert: FC1 (relu, bias via ACT), FC2
     (tanh via ACT + exp(10*t) via ACT), gate broadcast via K=1 ones-matmul,
     multiply-accumulate partial^T [O, B] (DVE).
  4. ReduceScatter(add) partial^T over 8 cores -> [O/8, B] shard, Ln, output.

Host unshard: concat o-shards -> [O, B], transpose -> [B, O].
"""

import ml_dtypes
import numpy as np

import concourse.bass as bass
import concourse.mybir as mybir
import concourse.tile as tile
from concourse import bacc
from concourse.alu_op_type import AluOpType
from concourse.bass_utils import run_bass_kernel_spmd
from concourse.masks import make_identity

F32 = mybir.dt.float32
F32R = mybir.dt.float32r  # 1 cycle/row on PE for N>=256 (vs 4 for fp32)
BF16 = mybir.dt.bfloat16
AF = mybir.ActivationFunctionType

B, I, H, O, E = 4096, 512, 1024, 512, 16
NCORES = 8
EL = E // NCORES          # 2 local experts per core
TOK = 512                 # token block (fp32 moving-operand max)
NBLK = B // TOK           # 8
KI = I // 128             # 4
KH = H // 128             # 8
KO = O // 128             # 4
NTT = B // 128            # 32 token tiles
TPB = TOK // 128          # 4 token tiles per block
OSH = O // NCORES         # 64 output rows per core after ReduceScatter

_BIG = 1.0e30


def _build_program():
    nc = bacc.Bacc(trn_type="TRN2", num_devices=NCORES)

    x_d = nc.dram_tensor("x", [B, I], F32, kind="ExternalInput")
    wg_d = nc.dram_tensor("wg", [I, E], F32, kind="ExternalInput")
    w1_d = nc.dram_tensor("w1", [EL, I, H], F32, kind="ExternalInput")
    b1_d = nc.dram_tensor("b1", [EL, H], F32, kind="ExternalInput")
    w2_d = nc.dram_tensor("w2", [EL, H, O], F32, kind="ExternalInput")
    b2_d = nc.dram_tensor("b2", [EL, O], F32, kind="ExternalInput")
    out_d = nc.dram_tensor("out", [OSH, B], F32, kind="ExternalOutput")

    with tile.TileContext(nc) as tc:
        with (
            tc.tile_pool(name="const", bufs=1) as constp,
            tc.tile_pool(name="xtp", bufs=1) as xtp,
            tc.tile_pool(name="wp", bufs=1) as wp,
            tc.tile_pool(name="gatep", bufs=1) as gatep,
            tc.tile_pool(name="dram", bufs=1, space="DRAM") as dramp,
        ):
            ident = constp.tile([128, 128], F32)
            make_identity(nc, ident[:])
            ones1f = constp.tile([1, 128], F32)
            nc.vector.memset(ones1f[:], 1.0)
            ones1 = constp.tile([1, 128], F32R)
            nc.vector.tensor_copy(ones1[:], ones1f[:])

            # ---------- weights (resident, bf16 from host) ----------
            # scalar-engine DMA queue so x tiles (sync queue) aren't stuck
            # behind the big weight transfers; per-expert chunks so expert 0's
            # FC1 can start as soon as its slice lands
            w1s = wp.tile([128, EL, KI, H], F32R)  # w1s[p,e,ki,h] = W1[e, ki*128+p, h]
            w2s = wp.tile([128, EL, KH, O], F32R)  # w2s[p,e,kh,o] = W2[e, kh*128+p, o]
            for e in range(EL):
                nc.scalar.dma_start(
                    w1s[:, e], w1_d[e].rearrange("(ki p) h -> p ki h", p=128).bitcast(F32R)
                )
                nc.scalar.dma_start(
                    w2s[:, e], w2_d[e].rearrange("(kh p) o -> p kh o", p=128).bitcast(F32R)
                )
            b1T = wp.tile([128, EL, KH], F32)     # b1T[p,e,hi] = b1[e, hi*128+p]
            nc.scalar.dma_start(b1T[:], b1_d.rearrange("e (kh p) -> p e kh", p=128))
            b2T = wp.tile([128, EL, KO], F32)
            nc.scalar.dma_start(b2T[:], b2_d.rearrange("e (ko p) -> p e ko", p=128))

            # persistent xT and local gates
            xT = []
            for ki in range(KI):
                t_ = xtp.tile([128, B], F32R, name=f"xT{ki}")
                xT.append(t_)
            gloc = gatep.tile([128, NTT, EL], F32)

            # ---------- scoped: x transpose + gating ----------
            with (
                tc.tile_pool(name="scratch", bufs=1) as scr,
                tc.tile_pool(name="psum_s", bufs=2, space="PSUM") as psum_s,
            ):
                wgs = scr.tile([128, KI, E], F32)  # wgs[p,ki,e] = wg[ki*128+p, e]
                nc.sync.dma_start(wgs[:], wg_d.rearrange("(ki p) e -> p ki e", p=128))

                # logits, token-major packed [128, NTT, E].  The gating matmul
                # must be EXACT fp32 (top-k flips are catastrophic), so evac
                # each transposed block twice: fp32r into resident xT for the
                # expert FCs, fp32 into a transient block for the logits.
                Lg = scr.tile([128, NTT, E], F32)
                for t in range(NTT):
                    x_tile = scr.tile([128, I], F32, tag="x_in", bufs=3)
                    nc.sync.dma_start(x_tile[:], x_d[t * 128:(t + 1) * 128, :])
                    xtg = scr.tile([128, KI, 128], F32, tag="xtg", bufs=3)
                    for ki in range(KI):
                        pt = psum_s.tile([128, 128], F32, tag="ptr")
                        nc.tensor.transpose(
                            pt[:], x_tile[:, ki * 128:(ki + 1) * 128], ident[:]
                        )
                        if (t * KI + ki) % 2 == 0:
                            nc.scalar.activation(
                                xT[ki][:, t * 128:(t + 1) * 128], pt[:], AF.Copy
                            )
                            nc.vector.tensor_copy(xtg[:, ki, :], pt[:])
                        else:
                            nc.vector.tensor_copy(
                                xT[ki][:, t * 128:(t + 1) * 128], pt[:]
                            )
                            nc.scalar.activation(xtg[:, ki, :], pt[:], AF.Copy)
                    pg = psum_s.tile([128, E], F32, tag="pg")
                    for ki in range(KI):
                        nc.tensor.matmul(
                            pg[:],
                            xtg[:, ki, :],
                            wgs[:, ki, :],
                            start=(ki == 0),
                            stop=(ki == KI - 1),
                        )
                    nc.vector.tensor_copy(Lg[:, t, :], pg[:])

                # top-4 of 16 per token
                mx = [scr.tile([128, NTT, 1], F32, name=f"mx{j}") for j in range(4)]
                eq = [scr.tile([128, NTT, E], F32, name=f"eq{j}") for j in range(4)]
                for j in range(4):
                    nc.vector.tensor_reduce(
                        mx[j][:], Lg[:], mybir.AxisListType.X, AluOpType.max
                    )
                    nc.vector.tensor_tensor(
                        eq[j][:], Lg[:], mx[j].to_broadcast([128, NTT, E]),
                        AluOpType.is_equal,
                    )
                    if j < 3:
                        # Lg = (eq * -BIG) + Lg  -- knock out the found max
                        nc.vector.scalar_tensor_tensor(
                            Lg[:], eq[j][:], -_BIG, Lg[:],
                            AluOpType.mult, AluOpType.add,
                        )

                # softmax over the 4 maxima: g_j = exp(m_j - m_0) / sum
                ex = [scr.tile([128, NTT, 1], F32, name=f"ex{j}") for j in range(4)]
                for j in range(1, 4):
                    nc.vector.tensor_sub(ex[j][:], mx[j][:], mx[0][:])
                    nc.scalar.activation(ex[j][:], ex[j][:], AF.Exp)
                denom = scr.tile([128, NTT, 1], F32)
                nc.vector.tensor_add(denom[:], ex[1][:], ex[2][:])
                nc.vector.tensor_add(denom[:], denom[:], ex[3][:])
                nc.vector.tensor_scalar_add(denom[:], denom[:], 1.0)
                rec = scr.tile([128, NTT, 1], F32)
                nc.vector.reciprocal(rec[:], denom[:])
                gj = [scr.tile([128, NTT, 1], F32, name=f"gj{j}") for j in range(4)]
                nc.vector.tensor_copy(gj[0][:], rec[:])
                for j in range(1, 4):
                    nc.vector.tensor_mul(gj[j][:], ex[j][:], rec[:])

                # local dense gates (this core's experts are gate cols 0..EL-1)
                tmpg = scr.tile([128, NTT, EL], F32)
                nc.vector.tensor_tensor(
                    gloc[:], eq[0][:, :, :EL], gj[0].to_broadcast([128, NTT, EL]),
                    AluOpType.mult,
                )
                for j in range(1, 4):
                    nc.vector.tensor_tensor(
                        tmpg[:], eq[j][:, :, :EL], gj[j].to_broadcast([128, NTT, EL]),
                        AluOpType.mult,
                    )
                    nc.vector.tensor_add(gloc[:], gloc[:], tmpg[:])

            # ---------- main loop: MLP + combine ----------
            # asymmetric token split (6 blocks / 2 blocks): the big first
            # ReduceScatter overlaps the tail compute, the small second one
            # is the only serial tail
            BLKA = 6
            B_A = BLKA * TOK
            B_B = B - B_A
            partial_a = dramp.tile([O, B_A], F32)
            partial_b = dramp.tile([O, B_B], F32)
            pviews = [
                partial_a.rearrange("(oi p) b -> p oi b", p=128),
                partial_b.rearrange("(oi p) b -> p oi b", p=128),
            ]

            with (
                tc.tile_pool(name="work", bufs=2) as work,
                tc.tile_pool(name="psum_m", bufs=2, space="PSUM") as psum_m,
            ):
                for blk in range(NBLK):
                    t0 = blk * TOK
                    part_blk = work.tile([128, KO, TOK], F32, tag="part", bufs=2)
                    for e in range(EL):
                        # assemble gate row [1, TOK] via per-token-tile PE transpose
                        grow = work.tile([1, TOK], F32R, tag="grow", bufs=2)
                        for j in range(TPB):
                            ptg = psum_m.tile([1, 128], F32, tag="ptg")
                            nc.tensor.transpose(
                                ptg[:], gloc[:, blk * TPB + j, e:e + 1], ident[:]
                            )
                            nc.vector.tensor_copy(
                                grow[:, j * 128:(j + 1) * 128], ptg[:]
                            )
                        # broadcast gate row across 128 partitions (K=1 matmul)
                        pgb = psum_m.tile([128, TOK], F32, tag="pgb")
                        nc.tensor.matmul(
                            pgb[:], ones1[:], grow[:],
                            start=True, stop=True,
                        )

                        hT = work.tile([128, KH, TOK], F32R, tag="hT", bufs=2)
                        for hi in range(KH):
                            ph = psum_m.tile([128, TOK], F32, tag="ph")
                            for ki in range(KI):
                                nc.tensor.matmul(
                                    ph[:],
                                    w1s[:, e, ki, hi * 128:(hi + 1) * 128],
                                    xT[ki][:, t0:t0 + TOK],
                                    start=(ki == 0),
                                    stop=(ki == KI - 1),
                                )
                            nc.scalar.activation(
                                hT[:, hi, :], ph[:], AF.Relu,
                                bias=b1T[:, e, hi:hi + 1],
                            )
                        for oi in range(KO):
                            po = psum_m.tile([128, TOK], F32, tag="po")
                            for hi in range(KH):
                                nc.tensor.matmul(
                                    po[:],
                                    w2s[:, e, hi, oi * 128:(oi + 1) * 128],
                                    hT[:, hi, :],
                                    start=(hi == 0),
                                    stop=(hi == KH - 1),
                                )
                            tt = work.tile([128, TOK], F32, tag="tt", bufs=2)
                            nc.scalar.activation(
                                tt[:], po[:], AF.Tanh, bias=b2T[:, e, oi:oi + 1]
                            )
                            last_act = nc.scalar.activation(
                                tt[:], tt[:], AF.Exp, scale=10.0
                            )
                            # multiply by broadcast gate (read PSUM directly)
                            if e == 0:
                                nc.vector.tensor_tensor(
                                    part_blk[:, oi, :], tt[:], pgb[:],
                                    AluOpType.mult,
                                )
                            else:
                                nc.vector.tensor_tensor(
                                    tt[:], tt[:], pgb[:], AluOpType.mult
                                )
                                nc.vector.tensor_add(
                                    part_blk[:, oi, :], part_blk[:, oi, :], tt[:]
                                )
                    half = 0 if blk < BLKA else 1
                    th = t0 - half * B_A
                    last_pdma = nc.sync.dma_start(
                        pviews[half][:, :, th:th + TOK], part_blk[:]
                    )
                # Emit both collectives AFTER the block loop: the gpsimd queue
                # stalls at a collective until it completes, so any Tile
                # bookkeeping emitted later on gpsimd (which the PE stream can
                # depend on) must not sit behind it.  Deps still let RS-a
                # start as soon as the partial_a DMAs (block BLKA-1) land,
                # overlapping the remaining blocks' compute.
                rs_out_a = dramp.tile([OSH, B_A], F32)
                nc.gpsimd.collective_compute(
                    "ReduceScatter",
                    AluOpType.add,
                    replica_groups=[list(range(NCORES))],
                    ins=[partial_a[:]],
                    outs=[rs_out_a[:]],
                )

                rs_out_b = dramp.tile([OSH, B_B], F32)
                nc.gpsimd.collective_compute(
                    "ReduceScatter",
                    AluOpType.add,
                    replica_groups=[list(range(NCORES))],
                    ins=[partial_b[:]],
                    outs=[rs_out_b[:]],
                )

                # ---------- log + output ----------
                for off, width, rs_o, nm in (
                    (0, B_A, rs_out_a, "fina"),
                    (B_A, B_B, rs_out_b, "finb"),
                ):
                    fin = work.tile([OSH, width], F32, tag="part", name=nm)
                    fdma = nc.sync.dma_start(fin[:], rs_o[:])
                    ln = nc.scalar.activation(fin[:], fin[:], AF.Ln)
                    # ordering-only edges: these wait on the collective, so
                    # they must not be scheduled ahead of the tail blocks'
                    # work in the in-order ACT / sync queues
                    tile.add_dep_helper(
                        fdma.ins, last_pdma.ins, sync=False,
                        reason="tail fin DMA after last partial DMA",
                    )
                    tile.add_dep_helper(
                        ln.ins, last_act.ins, sync=False,
                        reason="tail Ln after last main-loop ACT",
                    )
                    nc.sync.dma_start(out_d[:, off:off + width], fin[:])

    nc.compile()
    return nc


_NC_CACHE = None
LAST_RESULT = None


def _get_nc():
    global _NC_CACHE
    if _NC_CACHE is None:
        _NC_CACHE = _build_program()
    return _NC_CACHE


def kernel(x, w_gate, W1, b1, W2, b2, k, trace=False):
    global LAST_RESULT
    assert int(k) == 4
    x = np.ascontiguousarray(np.asarray(x, dtype=np.float32))
    w_gate = np.asarray(w_gate, dtype=np.float32)
    W1 = np.asarray(W1, dtype=np.float32)
    b1 = np.asarray(b1, dtype=np.float32)
    W2 = np.asarray(W2, dtype=np.float32)
    b2 = np.asarray(b2, dtype=np.float32)

    nc = _get_nc()
    in_maps = []
    for c in range(NCORES):
        mine = [EL * c + j for j in range(EL)]
        rest = [e for e in range(E) if e not in mine]
        perm = mine + rest
        in_maps.append({
            "x": x,
            "wg": np.ascontiguousarray(w_gate[:, perm]),
            "w1": np.ascontiguousarray(W1[mine]),
            "b1": np.ascontiguousarray(b1[mine]),
            "w2": np.ascontiguousarray(W2[mine]),
            "b2": np.ascontiguousarray(b2[mine]),
        })

    res = run_bass_kernel_spmd(
        nc, in_maps, core_ids=list(range(NCORES)), trace=trace
    )
    LAST_RESULT = res
    outT = np.concatenate([res.results[c]["out"] for c in range(NCORES)], axis=0)
    return np.ascontiguousarray(outT.T)


# revision 35
# speedup vs baseline: 1.0929x; 1.0116x over previous
"""Trainium2 Bass kernel for MoE (noisy top-k gating, eval path) over 8 NeuronCores.

Strategy: expert-parallel. Each core owns E/8 = 2 experts (weights sharded on host).
Every core receives the full x and a per-core column-permuted w_gate so that its own
experts sit in gate columns 0..1 (top-k is permutation invariant). On device:

  1. Transpose x -> xT [I, B] via PE (128x128 identity matmuls).
  2. Gating: logits token-major via matmul, top-4 of 16 via 4x (reduce_max,
     is_equal, mask-subtract), softmax over the 4 maxima, local gate columns.
  3. Per token-block (512) x per local expert: FC1 (relu, bias via ACT), FC2
     (tanh via ACT + exp(10*t) via ACT), gate broadcast via K=1 ones-matmul,
     multiply-accumulate partial^T [O, B] (DVE).
  4. ReduceScatter(add) partial^T over 8 cores -> [O/8, B] shard, Ln, output.

Host unshard: concat o-shards -> [O, B], transpose -> [B, O].
"""

import ml_dtypes
import numpy as np

import concourse.bass as bass
import concourse.mybir as mybir
import concourse.tile as tile
from concourse import bacc
from concourse.alu_op_type import AluOpType
from concourse.bass_utils import run_bass_kernel_spmd
from concourse.masks import make_identity

F32 = mybir.dt.float32
F32R = mybir.dt.float32r  # 1 cycle/row on PE for N>=256 (vs 4 for fp32)
BF16 = mybir.dt.bfloat16
AF = mybir.ActivationFunctionType

B, I, H, O, E = 4096, 512, 1024, 512, 16
NCORES = 8
EL = E // NCORES          # 2 local experts per core
TOK = 512                 # token block (fp32 moving-operand max)
NBLK = B // TOK           # 8
KI = I // 128             # 4
KH = H // 128             # 8
KO = O // 128             # 4
NTT = B // 128            # 32 token tiles
TPB = TOK // 128          # 4 token tiles per block
OSH = O // NCORES         # 64 output rows per core after ReduceScatter

_BIG = 1.0e30


def _build_program():
    nc = bacc.Bacc(trn_type="TRN2", num_devices=NCORES)

    x_d = nc.dram_tensor("x", [B, I], F32, kind="ExternalInput")
    wg_d = nc.dram_tensor("wg", [I, E], F32, kind="ExternalInput")
    w1_d = nc.dram_tensor("w1", [EL, I, H], F32, kind="ExternalInput")
    b1_d = nc.dram_tensor("b1", [EL, H], F32, kind="ExternalInput")
    w2_d = nc.dram_tensor("w2", [EL, H, O], F32, kind="ExternalInput")
    b2_d = nc.dram_tensor("b2", [EL, O], F32, kind="ExternalInput")
    out_d = nc.dram_tensor("out", [OSH, B], F32, kind="ExternalOutput")

    with tile.TileContext(nc) as tc:
        with (
            tc.tile_pool(name="const", bufs=1) as constp,
            tc.tile_pool(name="xtp", bufs=1) as xtp,
            tc.tile_pool(name="wp", bufs=1) as wp,
            tc.tile_pool(name="gatep", bufs=1) as gatep,
            tc.tile_pool(name="dram", bufs=1, space="DRAM") as dramp,
        ):
            ident = constp.tile([128, 128], F32)
            make_identity(nc, ident[:])
            ones1f = constp.tile([1, 128], F32)
            nc.vector.memset(ones1f[:], 1.0)
            ones1 = constp.tile([1, 128], F32R)
            nc.vector.tensor_copy(ones1[:], ones1f[:])

            # ---------- weights (resident, bf16 from host) ----------
            # scalar-engine DMA queue so x tiles (sync queue) aren't stuck
            # behind the big weight transfers; per-expert chunks so expert 0's
            # FC1 can start as soon as its slice lands
            w1s = wp.tile([128, EL, KI, H], F32R)  # w1s[p,e,ki,h] = W1[e, ki*128+p, h]
            w2s = wp.tile([128, EL, KH, O], F32R)  # w2s[p,e,kh,o] = W2[e, kh*128+p, o]
            for e in range(EL):
                nc.scalar.dma_start(
                    w1s[:, e], w1_d[e].rearrange("(ki p) h -> p ki h", p=128).bitcast(F32R)
                )
                nc.scalar.dma_start(
                    w2s[:, e], w2_d[e].rearrange("(kh p) o -> p kh o", p=128).bitcast(F32R)
                )
            b1T = wp.tile([128, EL, KH], F32)     # b1T[p,e,hi] = b1[e, hi*128+p]
            nc.scalar.dma_start(b1T[:], b1_d.rearrange("e (kh p) -> p e kh", p=128))
            b2T = wp.tile([128, EL, KO], F32)
            nc.scalar.dma_start(b2T[:], b2_d.rearrange("e (ko p) -> p e ko", p=128))

            # persistent xT and local gates
            xT = []
            for ki in range(KI):
                t_ = xtp.tile([128, B], F32R, name=f"xT{ki}")
                xT.append(t_)
            gloc = gatep.tile([128, NTT, EL], F32)

            # ---------- scoped: x transpose + gating ----------
            with (
                tc.tile_pool(name="scratch", bufs=1) as scr,
                tc.tile_pool(name="psum_s", bufs=2, space="PSUM") as psum_s,
            ):
                wgs = scr.tile([128, KI, E], F32)  # wgs[p,ki,e] = wg[ki*128+p, e]
                nc.sync.dma_start(wgs[:], wg_d.rearrange("(ki p) e -> p ki e", p=128))

                # logits, token-major packed [128, NTT, E].  The gating matmul
                # must be EXACT fp32 (top-k flips are catastrophic), so evac
                # each transposed block twice: fp32r into resident xT for the
                # expert FCs, fp32 into a transient block for the logits.
                Lg = scr.tile([128, NTT, E], F32)
                for t in range(NTT):
                    x_tile = scr.tile([128, I], F32, tag="x_in", bufs=3)
                    nc.sync.dma_start(x_tile[:], x_d[t * 128:(t + 1) * 128, :])
                    xtg = scr.tile([128, KI, 128], F32, tag="xtg", bufs=3)
                    for ki in range(KI):
                        pt = psum_s.tile([128, 128], F32, tag="ptr")
                        nc.tensor.transpose(
                            pt[:], x_tile[:, ki * 128:(ki + 1) * 128], ident[:]
                        )
                        if (t * KI + ki) % 2 == 0:
                            nc.scalar.activation(
                                xT[ki][:, t * 128:(t + 1) * 128], pt[:], AF.Copy
                            )
                            nc.vector.tensor_copy(xtg[:, ki, :], pt[:])
                        else:
                            nc.vector.tensor_copy(
                                xT[ki][:, t * 128:(t + 1) * 128], pt[:]
                            )
                            nc.scalar.activation(xtg[:, ki, :], pt[:], AF.Copy)
                    pg = psum_s.tile([128, E], F32, tag="pg")
                    for ki in range(KI):
                        nc.tensor.matmul(
                            pg[:],
                            xtg[:, ki, :],
                            wgs[:, ki, :],
                            start=(ki == 0),
                            stop=(ki == KI - 1),
                        )
                    nc.vector.tensor_copy(Lg[:, t, :], pg[:])

                # top-4 of 16 per token
                mx = [scr.tile([128, NTT, 1], F32, name=f"mx{j}") for j in range(4)]
                eq = [scr.tile([128, NTT, E], F32, name=f"eq{j}") for j in range(4)]
                for j in range(4):
                    nc.vector.tensor_reduce(
                        mx[j][:], Lg[:], mybir.AxisListType.X, AluOpType.max
                    )
                    nc.vector.tensor_tensor(
                        eq[j][:], Lg[:], mx[j].to_broadcast([128, NTT, E]),
                        AluOpType.is_equal,
                    )
                    if j < 3:
                        # Lg = (eq * -BIG) + Lg  -- knock out the found max
                        nc.vector.scalar_tensor_tensor(
                            Lg[:], eq[j][:], -_BIG, Lg[:],
                            AluOpType.mult, AluOpType.add,
                        )

                # softmax over the 4 maxima: g_j = exp(m_j - m_0) / sum
                ex = [scr.tile([128, NTT, 1], F32, name=f"ex{j}") for j in range(4)]
                for j in range(1, 4):
                    nc.vector.tensor_sub(ex[j][:], mx[j][:], mx[0][:])
                    nc.scalar.activation(ex[j][:], ex[j][:], AF.Exp)
                denom = scr.tile([128, NTT, 1], F32)
                nc.vector.tensor_add(denom[:], ex[1][:], ex[2][:])
                nc.vector.tensor_add(denom[:], denom[:], ex[3][:])
                nc.vector.tensor_scalar_add(denom[:], denom[:], 1.0)
                rec = scr.tile([128, NTT, 1], F32)
                nc.vector.reciprocal(rec[:], denom[:])
                gj = [scr.tile([128, NTT, 1], F32, name=f"gj{j}") for j in range(4)]
                nc.vector.tensor_copy(gj[0][:], rec[:])
                for j in range(1, 4):
                    nc.vector.tensor_mul(gj[j][:], ex[j][:], rec[:])

                # local dense gates (this core's experts are gate cols 0..EL-1)
                tmpg = scr.tile([128, NTT, EL], F32)
                nc.vector.tensor_tensor(
                    gloc[:], eq[0][:, :, :EL], gj[0].to_broadcast([128, NTT, EL]),
                    AluOpType.mult,
                )
                for j in range(1, 4):
                    nc.vector.tensor_tensor(
                        tmpg[:], eq[j][:, :, :EL], gj[j].to_broadcast([128, NTT, EL]),
                        AluOpType.mult,
                    )
                    nc.vector.tensor_add(gloc[:], gloc[:], tmpg[:])

            # ---------- main loop: MLP + combine ----------
            # asymmetric token split (6 blocks / 2 blocks): the big first
            # ReduceScatter overlaps the tail compute, the small second one
            # is the only serial tail
            BLKA = 6
            B_A = BLKA * TOK
            B_B = B - B_A
            partial_a = dramp.tile([O, B_A], F32)
            partial_b = dramp.tile([O, B_B], F32)
            pviews = [
                partial_a.rearrange("(oi p) b -> p oi b", p=128),
                partial_b.rearrange("(oi p) b -> p oi b", p=128),
            ]

            with (
                tc.tile_pool(name="work", bufs=2) as work,
                tc.tile_pool(name="psum_m", bufs=2, space="PSUM") as psum_m,
            ):
                for blk in range(NBLK):
                    t0 = blk * TOK
                    part_blk = work.tile([128, KO, TOK], F32, tag="part", bufs=2)
                    for e in range(EL):
                        # assemble gate row [1, TOK] via per-token-tile PE transpose
                        grow = work.tile([1, TOK], F32R, tag="grow", bufs=2)
                        for j in range(TPB):
                            ptg = psum_m.tile([1, 128], F32, tag="ptg")
                            nc.tensor.transpose(
                                ptg[:], gloc[:, blk * TPB + j, e:e + 1], ident[:]
                            )
                            nc.vector.tensor_copy(
                                grow[:, j * 128:(j + 1) * 128], ptg[:]
                            )
                        # broadcast gate row across 128 partitions (K=1 matmul)
                        pgb = psum_m.tile([128, TOK], F32, tag="pgb")
                        nc.tensor.matmul(
                            pgb[:], ones1[:], grow[:],
                            start=True, stop=True,
                        )

                        hT = work.tile([128, KH, TOK], F32R, tag="hT", bufs=2)
                        for hi in range(KH):
                            ph = psum_m.tile([128, TOK], F32, tag="ph")
                            for ki in range(KI):
                                nc.tensor.matmul(
                                    ph[:],
                                    w1s[:, e, ki, hi * 128:(hi + 1) * 128],
                                    xT[ki][:, t0:t0 + TOK],
                                    start=(ki == 0),
                                    stop=(ki == KI - 1),
                                )
                            nc.scalar.activation(
                                hT[:, hi, :], ph[:], AF.Relu,
                                bias=b1T[:, e, hi:hi + 1],
                            )
                        for oi in range(KO):
                            po = psum_m.tile([128, TOK], F32, tag="po")
                            for hi in range(KH):
                                nc.tensor.matmul(
                                    po[:],
                                    w2s[:, e, hi, oi * 128:(oi + 1) * 128],
                                    hT[:, hi, :],
                                    start=(hi == 0),
                                    stop=(hi == KH - 1),
                                )
                            tt = work.tile([128, TOK], F32, tag="tt", bufs=2)
                            nc.scalar.activation(
                                tt[:], po[:], AF.Tanh, bias=b2T[:, e, oi:oi + 1]
                            )
                            last_act = nc.scalar.activation(
                                tt[:], tt[:], AF.Exp, scale=10.0
                            )
                            # multiply by broadcast gate (read PSUM directly)
                            if e == 0:
                                nc.vector.tensor_tensor(
                                    part_blk[:, oi, :], tt[:], pgb[:],
                                    AluOpType.mult,
                                )
                            else:
                                nc.vector.tensor_tensor(
                                    tt[:], tt[:], pgb[:], AluOpType.mult
                                )
                                nc.vector.tensor_add(
                                    part_blk[:, oi, :], part_blk[:, oi, :], tt[:]
                                )
                    half = 0 if blk < BLKA else 1
                    th = t0 - half * B_A
                    last_pdma = nc.sync.dma_start(
                        pviews[half][:, :, th:th + TOK], part_blk[:]
                    )
                    if blk == BLKA - 1:
                        # emit RS-a here so it starts as soon as partial_a is
                        # complete, overlapping the tail blocks' compute (the
                        # tail Ln/DMA ordering edges below keep the in-order
                        # engine queues from stalling on it)
                        rs_out_a = dramp.tile([OSH, B_A], F32)
                        nc.gpsimd.collective_compute(
                            "ReduceScatter",
                            AluOpType.add,
                            replica_groups=[list(range(NCORES))],
                            ins=[partial_a[:]],
                            outs=[rs_out_a[:]],
                        )
                rs_out_b = dramp.tile([OSH, B_B], F32)
                nc.gpsimd.collective_compute(
                    "ReduceScatter",
                    AluOpType.add,
                    replica_groups=[list(range(NCORES))],
                    ins=[partial_b[:]],
                    outs=[rs_out_b[:]],
                )

                # ---------- log + output ----------
                for off, width, rs_o, nm in (
                    (0, B_A, rs_out_a, "fina"),
                    (B_A, B_B, rs_out_b, "finb"),
                ):
                    fin = work.tile([OSH, width], F32, tag="part", name=nm)
                    fdma = nc.sync.dma_start(fin[:], rs_o[:])
                    ln = nc.scalar.activation(fin[:], fin[:], AF.Ln)
                    # ordering-only edges: these wait on the collective, so
                    # they must not be scheduled ahead of the tail blocks'
                    # work in the in-order ACT / sync queues
                    tile.add_dep_helper(
                        fdma.ins, last_pdma.ins, sync=False,
                        reason="tail fin DMA after last partial DMA",
                    )
                    tile.add_dep_helper(
                        ln.ins, last_act.ins, sync=False,
                        reason="tail Ln after last main-loop ACT",
                    )
                    nc.sync.dma_start(out_d[:, off:off + width], fin[:])

    nc.compile()
    return nc


_NC_CACHE = None
LAST_RESULT = None


def _get_nc():
    global _NC_CACHE
    if _NC_CACHE is None:
        _NC_CACHE = _build_program()
    return _NC_CACHE


def kernel(x, w_gate, W1, b1, W2, b2, k, trace=False):
    global LAST_RESULT
    assert int(k) == 4
    x = np.ascontiguousarray(np.asarray(x, dtype=np.float32))
    w_gate = np.asarray(w_gate, dtype=np.float32)
    W1 = np.asarray(W1, dtype=np.float32)
    b1 = np.asarray(b1, dtype=np.float32)
    W2 = np.asarray(W2, dtype=np.float32)
    b2 = np.asarray(b2, dtype=np.float32)

    nc = _get_nc()
    in_maps = []
    for c in range(NCORES):
        mine = [EL * c + j for j in range(EL)]
        rest = [e for e in range(E) if e not in mine]
        perm = mine + rest
        in_maps.append({
            "x": x,
            "wg": np.ascontiguousarray(w_gate[:, perm]),
            "w1": np.ascontiguousarray(W1[mine]),
            "b1": np.ascontiguousarray(b1[mine]),
            "w2": np.ascontiguousarray(W2[mine]),
            "b2": np.ascontiguousarray(b2[mine]),
        })

    res = run_bass_kernel_spmd(
        nc, in_maps, core_ids=list(range(NCORES)), trace=trace
    )
    LAST_RESULT = res
    outT = np.concatenate([res.results[c]["out"] for c in range(NCORES)], axis=0)
    return np.ascontiguousarray(outT.T)
